# revision 18
# baseline (speedup 1.0000x reference)
"""StyleGAN2 conv_downsample_2d (FIR [1,3,3,1] + strided 1x1 conv) on 8 TRN2 cores.

Math (NCHW, per sample n):
    out[co, i, j] = sum_ci w[ci,co] * sum_{dy,dx} K2D[dy,dx] * x[ci, 2i+dy-1, 2j+dx-1]
with K2D = outer(k,k)/64, k = [1,3,3,1]  (symmetric, so the spatial flip is a no-op).

The kernel is HBM-bandwidth bound, so all device IO is fp16: inputs are
quantized on the host, the output is written fp16 and upcast after gather
(quantization error ~1e-3 of absmax; accumulation stays fp32 in PSUM).

Decomposition per core (data-parallel over (sample, H-half) -> 8 shards):
  1. Vertical 4-tap FIR at row-stride 2 on VectorE:
       s = x1+x2 (2x mode), t = x0+x3 (2x), p = 3*s (4x), v = p+t (2x)
     (unnormalized; /64 and the horizontal taps are folded into the 1x1-conv
     weights on the host). Input tiles carry a 2-row halo so every chunk's
     adds are single full-height ops.
  2. Horizontal FIR + channel mix fused on TensorE: 4 PSUM-accumulating
     matmuls per output tile; tap dx selects a stride-2 column window of the
     interleaved v row; lhsT = w * k[dx]/64 (precomputed on host).
  3. PSUM -> SBUF (fp16) on ScalarE; output DMA issued from the ScalarE
     HWDGE ring so it never queues behind input DMAs on the Sync ring.

Each shard is host-padded to a uniform [128, 258, 512] row window so all 8
cores run the identical SPMD program (no partition-id branching).
"""

import numpy as np

import concourse.bass as bass
import concourse.mybir as mybir
from concourse import bacc
from concourse.tile import TileContext
from concourse.bass_utils import run_bass_kernel_spmd

N_CORES = 8
C_IN = 128
C_OUT = 256
H = 512
W = 512
HO = 256  # full output rows; 128 per core
WO = 256
SHARD_ROWS = 258  # 2*128 rows of taps + 2 boundary rows (host zero-padded)
TILE_ROWS = 34  # 32 fresh rows + 2-row halo; 8 uniform tiles cover all 258 rows
N_TILES = 8
N_CHUNKS = 16  # v-chunks of 8 output rows -> 128 output rows per core
VW = 516  # v row width: col1 = left zero pad, cols 2..513 real, col 514 = right pad
          # (516 keeps every row start 8B-aligned so 2x-mode writes stay 4B-aligned)

F16 = mybir.dt.float16
F32 = mybir.dt.float32

_CACHED_NC = None


def _build_program():
    nc = bacc.Bacc("TRN2", target_bir_lowering=False)

    x = nc.dram_tensor("x", [C_IN, SHARD_ROWS, W], F16, kind="ExternalInput")
    wp = nc.dram_tensor("wp", [C_IN, 4, 2, 128], F16, kind="ExternalInput")
    out = nc.dram_tensor("out", [C_OUT, HO // 2, WO], F16, kind="ExternalOutput")

    with TileContext(nc) as tc:
        with (
            tc.tile_pool(name="inp", bufs=4) as inp_pool,
            tc.tile_pool(name="vpool", bufs=2) as v_pool,
            tc.tile_pool(name="stpool", bufs=1) as st_pool,
            tc.tile_pool(name="stage", bufs=2) as stage_pool,
            tc.tile_pool(name="wpool", bufs=1) as w_pool,
            tc.tile_pool(name="psum", bufs=2, space="PSUM") as psum_pool,
        ):
            wsb = w_pool.tile([C_IN, 4, 2, 128], F16, tag="w")
            nc.sync.dma_start(out=wsb[:], in_=wp[:])

            # s/t/p scratch: fully rewritten every chunk (no carried state ->
            # no cross-chunk scheduling dependencies).
            s = st_pool.tile([C_IN, 8, W], F16, tag="s")
            t = st_pool.tile([C_IN, 8, W], F16, tag="t")
            p = st_pool.tile([C_IN, 8, W], F16, tag="p")

            tiles: dict[int, object] = {}

            def in_tile(k):
                # Tile k covers shard rows 32k .. 32k+33 (2-row halo); every
                # chunk reads one tile only. Each tile loads as two sub-DMAs
                # split at the even chunk's row window (rows 0..17 / 18..33)
                # so compute can start after the first ~2.25MB lands. Tile 0's
                # first window is split again so the kernel's very first
                # compute block only waits on a ~1.25MB transfer.
                if k not in tiles:
                    tl = inp_pool.tile([C_IN, TILE_ROWS, W], F16, tag="in")
                    r0 = 32 * k
                    if k == 0:
                        nc.sync.dma_start(out=tl[:, 0:10, :], in_=x[:, 0:10, :])
                        nc.sync.dma_start(out=tl[:, 10:18, :], in_=x[:, 10:18, :])
                    else:
                        nc.sync.dma_start(
                            out=tl[:, 0:18, :], in_=x[:, r0 : r0 + 18, :]
                        )
                    nc.sync.dma_start(
                        out=tl[:, 18:34, :], in_=x[:, r0 + 18 : r0 + 34, :]
                    )
                    tiles[k] = tl
                return tiles[k]

            # Emit every input load up front: the triggers depend only on
            # buffer rotation, so the scheduler can run the input stream
            # ahead of compute instead of pacing it chunk-by-chunk.
            for k in range(N_TILES):
                in_tile(k)

            # out viewed as [co_local=128, half, row, col] so one DMA can write
            # both co-halves of a chunk from a single stage tile.
            out_hv = out.rearrange("(h co) i j -> co h i j", h=2)

            def emit_block(vrow0, nrows, ta, roff):
                """One v-block: v rows [vrow0, vrow0+nrows). Tap m (0..nrows)
                reads tile `ta` local rows roff+2m .. roff+2m+3 (the 2-row
                halo guarantees roff+2*nrows+1 <= 33)."""
                v = v_pool.tile([C_IN, nrows, VW], F16, tag="v")

                # s[m] = x[2m+1] + x[2m+2]   (middle taps, weight 3)
                # t[m] = x[2m] + x[2m+3]     (outer taps, weight 1)
                nc.vector.tensor_add(
                    out=s[:, 0:nrows, :],
                    in0=ta[:, roff + 1 : roff + 2 * nrows : 2, :],
                    in1=ta[:, roff + 2 : roff + 2 * nrows + 1 : 2, :],
                )
                nc.vector.tensor_add(
                    out=t[:, 0:nrows, :],
                    in0=ta[:, roff : roff + 2 * nrows - 1 : 2, :],
                    in1=ta[:, roff + 3 : roff + 2 * nrows + 2 : 2, :],
                )
                # v[m, 2+c] = 3*s[m,c] + t[m,c] as two 2x/4x-mode ops (the
                # fused scalar_tensor_tensor only has a 1x uop; GpSimd measures
                # ~50x slower here and stalls DVE via the shared SBUF port).
                nc.vector.tensor_scalar_mul(p[:, 0:nrows, :], s[:, 0:nrows, :], 3.0)
                nc.vector.tensor_add(
                    out=v[:, 0:nrows, 2 : 2 + W],
                    in0=p[:, 0:nrows, :],
                    in1=t[:, 0:nrows, :],
                )

                # Horizontal FIR + 1x1 conv: out[co, m, j] = sum_dx lhsT_dx.T
                # @ v[., 2j+dx+1] with v cols 2..513 real. The edge taps
                # (dx=0 at j=0, dx=3 at j=255) would read zero-pad cells; the
                # pad is never materialized — those taps just use a clipped
                # column range, and PSUM has_written turns the first actual
                # write to an uncovered column into an overwrite.
                # rhs column window and psum column offset per tap:
                TAPS = [(3, 1, 255), (2, 0, 256), (3, 0, 256), (4, 0, 255)]
                # Both co-halves land in one stage tile so the chunk's output
                # leaves in a single 1MB DMA (fewer HBM write turnarounds).
                stage = stage_pool.tile([128, 2, nrows, WO], F16, tag="stage")
                for half in range(2):
                    # One multi-bank PSUM tile per half: each row-pair's 4-tap
                    # accumulation group lands in its own (bank-aligned) 2KB
                    # slice, and the whole tile drains with a single ACT copy
                    # (per-op bubble would dominate with per-bank copies).
                    ps = psum_pool.tile([128, nrows, WO], F32, tag="ps")
                    for rp in range(nrows // 2):
                        for dx in range(4):
                            c0, j0, nj = TAPS[dx]
                            nc.tensor.matmul(
                                ps[:, 2 * rp : 2 * rp + 2, j0 : j0 + nj],
                                wsb[:, dx, half, :],
                                v[:, 2 * rp : 2 * rp + 2, c0 : c0 + 2 * nj : 2],
                                start=(dx == 0),
                                stop=(dx == 3),
                            )
                    nc.scalar.copy(out=stage[:, half], in_=ps[:])
                nc.scalar.dma_start(
                    out=out_hv[:, :, vrow0 : vrow0 + nrows, :],
                    in_=stage[:],
                )

            # Chunk 0 runs as two 4-row blocks so the first compute only
            # depends on tile 0's first 10-row sub-DMA.
            emit_block(0, 4, in_tile(0), 0)
            emit_block(4, 4, in_tile(0), 8)
            for c in range(1, N_CHUNKS - 2):
                # chunk c needs shard rows 16c..16c+17: tile c//2 rows
                # 16(c%2) .. 16(c%2)+17 (halo makes the odd chunk fit).
                emit_block(8 * c, 8, in_tile(c // 2), 16 * (c % 2))
            # Split the final two chunks into 4-row blocks so their outputs
            # stream out while later blocks compute — shortens the
            # end-of-kernel drain after the input stream finishes.
            for c in (N_CHUNKS - 2, N_CHUNKS - 1):
                roff = 16 * (c % 2)
                emit_block(8 * c, 4, in_tile(c // 2), roff)
                emit_block(8 * c + 4, 4, in_tile(c // 2), roff + 8)
    nc.finalize()
    return nc


def _get_nc():
    global _CACHED_NC
    if _CACHED_NC is None:
        _CACHED_NC = _build_program()
    return _CACHED_NC


def _prep_inputs(images, w):
    images = np.asarray(images, dtype=np.float32)
    w = np.asarray(w, dtype=np.float32)
    assert images.shape == (4, C_IN, H, W), images.shape
    assert w.shape == (1, 1, C_IN, C_OUT), w.shape

    k = np.array([1.0, 3.0, 3.0, 1.0], dtype=np.float32)
    # wq[ci, dx, half, co] = w[ci, 128*half+co] * k[dx] / 64
    wq = np.ascontiguousarray(
        w[0, 0].reshape(C_IN, 1, 2, 128) * (k / 64.0).reshape(1, 4, 1, 1)
    ).astype(np.float16)

    imh = images.astype(np.float16)
    zrow = np.zeros((C_IN, 1, W), dtype=np.float16)
    in_maps = []
    for n in range(4):
        # half 0: padded global rows -1..256 ; half 1: padded global rows 255..512
        shard0 = np.ascontiguousarray(
            np.concatenate([zrow, imh[n][:, 0:257, :]], axis=1)
        )
        shard1 = np.ascontiguousarray(
            np.concatenate([imh[n][:, 255:512, :], zrow], axis=1)
        )
        in_maps.append({"x": shard0, "wp": wq})
        in_maps.append({"x": shard1, "wp": wq})
    return in_maps


def _assemble(results):
    out = np.empty((4, C_OUT, HO, WO), dtype=np.float32)
    for n in range(4):
        for half in range(2):
            out[n, :, 128 * half : 128 * (half + 1), :] = np.asarray(
                results[2 * n + half]["out"], dtype=np.float32
            )
    return out


def run(images, w, **spmd_kwargs):
    """Full pipeline; returns (output, BassKernelResults)."""
    nc = _get_nc()
    in_maps = _prep_inputs(images, w)
    res = run_bass_kernel_spmd(nc, in_maps, core_ids=list(range(N_CORES)), **spmd_kwargs)
    return _assemble(res.results), res


def kernel(images, w):
    out, _ = run(images, w)
    return out


# revision 21
# speedup vs baseline: 1.0568x; 1.0568x over previous
"""StyleGAN2 conv_downsample_2d (FIR [1,3,3,1] + strided 1x1 conv) on 8 TRN2 cores.

Math (NCHW, per sample n):
    out[co, i, j] = sum_ci w[ci,co] * sum_{dy,dx} K2D[dy,dx] * x[ci, 2i+dy-1, 2j+dx-1]
with K2D = outer(k,k)/64, k = [1,3,3,1]  (symmetric, so the spatial flip is a no-op).

The kernel is HBM-bandwidth bound, so all device IO is fp16: inputs are
quantized on the host, the output is written fp16 and upcast after gather
(quantization error ~1e-3 of absmax; accumulation stays fp32 in PSUM).

Decomposition per core (data-parallel over (sample, H-half) -> 8 shards):
  1. Vertical 4-tap FIR at row-stride 2 on VectorE:
       s = x1+x2 (2x mode), t = x0+x3 (2x), p = 3*s (4x), v = p+t (2x)
     (unnormalized; /64 and the horizontal taps are folded into the 1x1-conv
     weights on the host). Input tiles carry a 2-row halo so every chunk's
     adds are single full-height ops.
  2. Horizontal FIR + channel mix fused on TensorE: 4 PSUM-accumulating
     matmuls per output tile; tap dx selects a stride-2 column window of the
     interleaved v row; lhsT = w * k[dx]/64 (precomputed on host).
  3. PSUM -> SBUF (fp16) on ScalarE; output DMA issued from the ScalarE
     HWDGE ring so it never queues behind input DMAs on the Sync ring.

Each shard is host-padded to a uniform [128, 258, 512] row window so all 8
cores run the identical SPMD program (no partition-id branching).
"""

import numpy as np

import concourse.bass as bass
import concourse.mybir as mybir
from concourse import bacc
from concourse.tile import TileContext
from concourse.bass_utils import run_bass_kernel_spmd

N_CORES = 8
C_IN = 128
C_OUT = 256
H = 512
W = 512
HO = 256  # full output rows; 128 per core
WO = 256
SHARD_ROWS = 258  # 2*128 rows of taps + 2 boundary rows (host zero-padded)
TILE_ROWS = 34  # 32 fresh rows + 2-row halo; 8 uniform tiles cover all 258 rows
N_TILES = 8
N_CHUNKS = 16  # v-chunks of 8 output rows -> 128 output rows per core
VW = 516  # v row width: col1 = left zero pad, cols 2..513 real, col 514 = right pad
          # (516 keeps every row start 8B-aligned so 2x-mode writes stay 4B-aligned)

F16 = mybir.dt.float16
F32 = mybir.dt.float32

_CACHED_NC = None


def _build_program():
    nc = bacc.Bacc("TRN2", target_bir_lowering=False)

    x = nc.dram_tensor("x", [C_IN, SHARD_ROWS, W], F16, kind="ExternalInput")
    wp = nc.dram_tensor("wp", [C_IN, 4, 2, 128], F16, kind="ExternalInput")
    out = nc.dram_tensor("out", [C_OUT, HO // 2, WO], F16, kind="ExternalOutput")

    with TileContext(nc) as tc:
        with (
            tc.tile_pool(name="inp", bufs=4) as inp_pool,
            tc.tile_pool(name="vpool", bufs=3) as v_pool,
            tc.tile_pool(name="stpool", bufs=1) as st_pool,
            tc.tile_pool(name="stage", bufs=2) as stage_pool,
            tc.tile_pool(name="wpool", bufs=1) as w_pool,
            tc.tile_pool(name="psum", bufs=2, space="PSUM") as psum_pool,
        ):
            wsb = w_pool.tile([C_IN, 4, 2, 128], F16, tag="w")
            nc.sync.dma_start(out=wsb[:], in_=wp[:])

            # s/t/p scratch: fully rewritten every chunk (no carried state ->
            # no cross-chunk scheduling dependencies).
            s = st_pool.tile([C_IN, 8, W], F16, tag="s")
            t = st_pool.tile([C_IN, 8, W], F16, tag="t")
            p = st_pool.tile([C_IN, 8, W], F16, tag="p")

            tiles: dict[int, object] = {}

            def in_tile(k):
                # Tile k covers shard rows 32k .. 32k+33 (2-row halo); every
                # chunk reads one tile only. Each tile loads as two sub-DMAs
                # split at the even chunk's row window (rows 0..17 / 18..33)
                # so compute can start after the first ~2.25MB lands. Tile 0's
                # first window is split again so the kernel's very first
                # compute block only waits on a ~1.25MB transfer.
                if k not in tiles:
                    tl = inp_pool.tile([C_IN, TILE_ROWS, W], F16, tag="in")
                    r0 = 32 * k
                    if k == 0:
                        nc.sync.dma_start(out=tl[:, 0:10, :], in_=x[:, 0:10, :])
                        nc.sync.dma_start(out=tl[:, 10:18, :], in_=x[:, 10:18, :])
                    else:
                        nc.sync.dma_start(
                            out=tl[:, 0:18, :], in_=x[:, r0 : r0 + 18, :]
                        )
                    nc.sync.dma_start(
                        out=tl[:, 18:34, :], in_=x[:, r0 + 18 : r0 + 34, :]
                    )
                    tiles[k] = tl
                return tiles[k]

            # Emit every input load up front: the triggers depend only on
            # buffer rotation, so the scheduler can run the input stream
            # ahead of compute instead of pacing it chunk-by-chunk.
            for k in range(N_TILES):
                in_tile(k)

            # out viewed as [co_local=128, half, row, col] so one DMA can write
            # both co-halves of a chunk from a single stage tile.
            out_hv = out.rearrange("(h co) i j -> co h i j", h=2)

            def emit_block(vrow0, nrows, ta, roff):
                """One v-block: v rows [vrow0, vrow0+nrows). Tap m (0..nrows)
                reads tile `ta` local rows roff+2m .. roff+2m+3 (the 2-row
                halo guarantees roff+2*nrows+1 <= 33)."""
                v = v_pool.tile([C_IN, nrows, VW], F16, tag="v")

                # s[m] = x[2m+1] + x[2m+2]   (middle taps, weight 3)
                # t[m] = x[2m] + x[2m+3]     (outer taps, weight 1)
                nc.vector.tensor_add(
                    out=s[:, 0:nrows, :],
                    in0=ta[:, roff + 1 : roff + 2 * nrows : 2, :],
                    in1=ta[:, roff + 2 : roff + 2 * nrows + 1 : 2, :],
                )
                nc.vector.tensor_add(
                    out=t[:, 0:nrows, :],
                    in0=ta[:, roff : roff + 2 * nrows - 1 : 2, :],
                    in1=ta[:, roff + 3 : roff + 2 * nrows + 2 : 2, :],
                )
                # v[m, 2+c] = 3*s[m,c] + t[m,c] as two 2x/4x-mode ops (the
                # fused scalar_tensor_tensor only has a 1x uop; GpSimd measures
                # ~50x slower here and stalls DVE via the shared SBUF port).
                nc.vector.tensor_scalar_mul(p[:, 0:nrows, :], s[:, 0:nrows, :], 3.0)
                nc.vector.tensor_add(
                    out=v[:, 0:nrows, 2 : 2 + W],
                    in0=p[:, 0:nrows, :],
                    in1=t[:, 0:nrows, :],
                )
                # zero-pad edge cells (cols 1 and 514); cols 0/515 are
                # alignment filler, never read. (Clipping the edge taps'
                # column ranges instead measures slower: N=510 matmuls run
                # ~14% worse per column than N=512.)
                nc.vector.tensor_scalar_mul(
                    v[:, 0:nrows, 1 : VW - 1 : VW - 3],
                    ta[:, 0:nrows, 0 : W : W - 1],
                    0.0,
                )

                # Horizontal FIR + 1x1 conv: out[co, m, j] = sum_dx lhsT_dx.T @ v[., 2j+dx+1]
                # Both co-halves land in one stage tile so the chunk's output
                # leaves in a single 1MB DMA (fewer HBM write turnarounds).
                stage = stage_pool.tile([128, 2, nrows, WO], F16, tag="stage")
                for half in range(2):
                    # One multi-bank PSUM tile per half: each row-pair's 4-tap
                    # accumulation group lands in its own (bank-aligned) 2KB
                    # slice, and the whole tile drains with a single ACT copy
                    # (per-op bubble would dominate with per-bank copies).
                    ps = psum_pool.tile([128, nrows, WO], F32, tag="ps")
                    for rp in range(nrows // 2):
                        for dx in range(4):
                            nc.tensor.matmul(
                                ps[:, 2 * rp : 2 * rp + 2, :],
                                wsb[:, dx, half, :],
                                v[:, 2 * rp : 2 * rp + 2, dx + 1 : dx + 1 + 2 * WO : 2],
                                start=(dx == 0),
                                stop=(dx == 3),
                            )
                    nc.scalar.copy(out=stage[:, half], in_=ps[:])
                nc.scalar.dma_start(
                    out=out_hv[:, :, vrow0 : vrow0 + nrows, :],
                    in_=stage[:],
                )

            # Chunk 0 runs as two 4-row blocks so the first compute only
            # depends on tile 0's first 10-row sub-DMA.
            emit_block(0, 4, in_tile(0), 0)
            emit_block(4, 4, in_tile(0), 8)
            for c in range(1, N_CHUNKS - 1):
                # chunk c needs shard rows 16c..16c+17: tile c//2 rows
                # 16(c%2) .. 16(c%2)+17 (halo makes the odd chunk fit).
                emit_block(8 * c, 8, in_tile(c // 2), 16 * (c % 2))
            # Split the final chunk into two 4-row blocks so its first half's
            # outputs stream out while the second half computes — shortens the
            # end-of-kernel drain after the input stream finishes.
            last = N_CHUNKS - 1
            emit_block(8 * last, 4, in_tile(last // 2), 16)
            emit_block(8 * last + 4, 4, in_tile(last // 2), 24)
    nc.finalize()
    return nc


def _get_nc():
    global _CACHED_NC
    if _CACHED_NC is None:
        _CACHED_NC = _build_program()
    return _CACHED_NC


def _prep_inputs(images, w):
    images = np.asarray(images, dtype=np.float32)
    w = np.asarray(w, dtype=np.float32)
    assert images.shape == (4, C_IN, H, W), images.shape
    assert w.shape == (1, 1, C_IN, C_OUT), w.shape

    k = np.array([1.0, 3.0, 3.0, 1.0], dtype=np.float32)
    # wq[ci, dx, half, co] = w[ci, 128*half+co] * k[dx] / 64
    wq = np.ascontiguousarray(
        w[0, 0].reshape(C_IN, 1, 2, 128) * (k / 64.0).reshape(1, 4, 1, 1)
    ).astype(np.float16)

    imh = images.astype(np.float16)
    zrow = np.zeros((C_IN, 1, W), dtype=np.float16)
    in_maps = []
    for n in range(4):
        # half 0: padded global rows -1..256 ; half 1: padded global rows 255..512
        shard0 = np.ascontiguousarray(
            np.concatenate([zrow, imh[n][:, 0:257, :]], axis=1)
        )
        shard1 = np.ascontiguousarray(
            np.concatenate([imh[n][:, 255:512, :], zrow], axis=1)
        )
        in_maps.append({"x": shard0, "wp": wq})
        in_maps.append({"x": shard1, "wp": wq})
    return in_maps


def _assemble(results):
    out = np.empty((4, C_OUT, HO, WO), dtype=np.float32)
    for n in range(4):
        for half in range(2):
            out[n, :, 128 * half : 128 * (half + 1), :] = np.asarray(
                results[2 * n + half]["out"], dtype=np.float32
            )
    return out


def run(images, w, **spmd_kwargs):
    """Full pipeline; returns (output, BassKernelResults)."""
    nc = _get_nc()
    in_maps = _prep_inputs(images, w)
    res = run_bass_kernel_spmd(nc, in_maps, core_ids=list(range(N_CORES)), **spmd_kwargs)
    return _assemble(res.results), res


def kernel(images, w):
    out, _ = run(images, w)
    return out
